# revision 5
# baseline (speedup 1.0000x reference)
"""GAT (2-layer graph attention network) Trainium2 kernel.

Contract: kernel(**inputs) takes the FULL inputs from setup_inputs() and
returns the full (32, 256, 512) float32 output. Internally shards the batch
across 8 NeuronCores (4 graphs per core), runs a Bass/Tile kernel per core,
and concatenates the results.

Math notes (reference in reference.py):
  x = embed[fea]                       -> computed on device as onehot @ embed
  Layer1 per head h: Wh = x @ W[h];  e1 = Wh @ a1 = x @ (W[h] @ a1)
    so [Wh | e1 | e2] come from ONE matmul with rhs [W[h] | W@a1 | W@a2].
  e = leaky_relu(e1[:,None] + e2[None,:], 0.2); mask; softmax; out = attn @ Wh.
  Softmax is computed without max-subtraction: exp values are bounded (e is
  O(1) for real rows; masked entries get -1000 added pre-leaky so
  exp(0.2*(-1000+e)) underflows to 0, matching where(mask, e, -9e15)).
  Layer1 output is produced o-major (Wh.T @ pT) which is exactly the hT
  layout layer 2 needs as its stationary operand - no h transpose.
  non_pad_mask folds into layer-2's PSUM->SBUF copy scale and the final
  activation scale (multiplying rows of a matmul output == masking its input).
"""

import numpy as np
from contextlib import ExitStack

import concourse.bass as bass
import concourse.tile as tile
from concourse import mybir, bacc
from concourse.bass_utils import run_bass_kernel_spmd

f32 = mybir.dt.float32
i32 = mybir.dt.int32

# Problem dims (hardcoded per contract)
B, N, VOCAB, F, O, H, OUT = 32, 256, 200, 300, 256, 8, 512
NCORES = 8
GPC = B // NCORES          # graphs per core
NC = N // 128              # node chunks (2)
FC = 3                     # feature chunks (F padded 300->384)
VC = 2                     # vocab chunks (padded 200->256)
KC2 = (H * O) // 128       # layer-2 contraction chunks (16)
ALPHA = 0.2
MASK_NEG = -1000.0

# Matmul dtype: float32 (exact, 4 cyc/row) or float32r (tf32-like, 1 cyc/row)
MM_DT = mybir.dt.float32r


def _build_nc(mm_dt):
    nc = bacc.Bacc("TRN2", target_bir_lowering=False, debug=False,
                   num_devices=NCORES)

    # --- DRAM tensors (per-core shapes; host pre-layouts for clean DMA) ---
    oh_d = nc.dram_tensor("oh", [GPC, 128, VC, N], f32, kind="ExternalInput").ap()
    adj_d = nc.dram_tensor("adjm", [GPC, 128, NC, N], i32, kind="ExternalInput").ap()
    npm_d = nc.dram_tensor("npm", [GPC, 128, NC], f32, kind="ExternalInput").ap()
    emb_d = nc.dram_tensor("emb", [VC, 128, 384], f32, kind="ExternalInput").ap()
    w1_d = nc.dram_tensor("w1aug", [H, FC, 128, O + 2], f32, kind="ExternalInput").ap()
    w2_d = nc.dram_tensor("w2aug", [KC2, 128, OUT + 2], f32, kind="ExternalInput").ap()
    idn_d = nc.dram_tensor("identity", [128, 128], f32, kind="ExternalInput").ap()
    out_d = nc.dram_tensor("out", [GPC, 128, NC, OUT], f32, kind="ExternalOutput").ap()

    with tile.TileContext(nc) as tc, ExitStack() as ctx:
        const = ctx.enter_context(tc.tile_pool(name="const", bufs=1))
        stage = ctx.enter_context(tc.tile_pool(name="stage", bufs=2))
        gpool = ctx.enter_context(tc.tile_pool(name="gpool", bufs=2))
        hpool = ctx.enter_context(tc.tile_pool(name="hpool", bufs=3))
        hbig = ctx.enter_context(tc.tile_pool(name="hbig", bufs=1))
        drp = ctx.enter_context(tc.tile_pool(name="drp", bufs=4, space="DRAM"))
        ps_aug = ctx.enter_context(tc.tile_pool(name="ps_aug", bufs=2, space="PSUM"))
        ps_big = ctx.enter_context(tc.tile_pool(name="ps_big", bufs=2, space="PSUM"))
        ps_tr = ctx.enter_context(tc.tile_pool(name="ps_tr", bufs=4, space="PSUM"))

        def round_copy(dst, src):
            # produce a tile consumable by an mm_dt matmul
            nc.vector.tensor_copy(dst, src)

        ident = const.tile([128, 128], f32)
        nc.sync.dma_start(ident[:], idn_d)

        # ---- resident weights (rounded to mm_dt if needed) ----
        emb_sb = const.tile([128, VC, 384], mm_dt)
        w1_sb = const.tile([128, H, FC, O + 2], mm_dt)
        w2_sb = const.tile([128, KC2, OUT + 2], mm_dt)
        if mm_dt == f32:
            for c in range(VC):
                nc.sync.dma_start(emb_sb[:, c, :], emb_d[c])
            for h in range(H):
                for k in range(FC):
                    nc.sync.dma_start(w1_sb[:, h, k, :], w1_d[h, k])
            for k in range(KC2):
                nc.sync.dma_start(w2_sb[:, k, :], w2_d[k])
        else:
            for c in range(VC):
                st = stage.tile([128, 384], f32, tag="wstage")
                nc.sync.dma_start(st[:], emb_d[c])
                round_copy(emb_sb[:, c, :], st[:])
            for h in range(H):
                for k in range(FC):
                    st = stage.tile([128, O + 2], f32, tag="wstage")
                    nc.sync.dma_start(st[:], w1_d[h, k])
                    round_copy(w1_sb[:, h, k, :], st[:])
            for k in range(KC2):
                st = stage.tile([128, OUT + 2], f32, tag="wstage")
                nc.sync.dma_start(st[:], w2_d[k])
                round_copy(w2_sb[:, k, :], st[:])

        for g in range(GPC):
            # ---- per-graph inputs ----
            oh_f = gpool.tile([128, VC, N], f32)
            nc.sync.dma_start(oh_f[:], oh_d[g])
            if mm_dt == f32:
                oh_sb = oh_f
            else:
                oh_sb = gpool.tile([128, VC, N], mm_dt)
                round_copy(oh_sb[:], oh_f[:])
            adj_sb = gpool.tile([128, NC, N], i32)
            nc.sync.dma_start(adj_sb[:], adj_d[g])
            npm_sb = gpool.tile([128, NC], f32)
            nc.sync.dma_start(npm_sb[:], npm_d[g])
            # mneg[n, m] = 0 where edge, MASK_NEG where not
            mneg = gpool.tile([128, NC, N], f32)
            nc.vector.tensor_scalar(
                mneg[:], adj_sb[:], 1.0, -MASK_NEG,
                op0=mybir.AluOpType.subtract, op1=mybir.AluOpType.mult,
            )

            # ---- xT = (onehot @ embed).T : [f, n] f-major via PE ----
            xT = gpool.tile([128, FC, N], mm_dt)
            for fc in range(FC):
                ps = ps_big.tile([128, N], f32, tag="big")
                for vc in range(VC):
                    nc.tensor.matmul(
                        ps[:], lhsT=emb_sb[:, vc, fc * 128:(fc + 1) * 128],
                        rhs=oh_sb[:, vc, :],
                        start=(vc == 0), stop=(vc == VC - 1),
                    )
                nc.scalar.copy(xT[:, fc, :], ps[:])

            # ---- layer 1: hT [hf, n] assembled o-major ----
            hT = hbig.tile([128, KC2, N], mm_dt)
            for h in range(H):
                # augmented matmul: [Wh | e1 | e2] per node chunk
                wh_sb = hpool.tile([128, NC, O + 2], mm_dt)
                e2row = drp.tile([N], f32)
                for c in range(NC):
                    aug = ps_aug.tile([128, O + 2], f32, tag="aug")
                    for fc in range(FC):
                        nc.tensor.matmul(
                            aug[:], lhsT=xT[:, fc, c * 128:(c + 1) * 128],
                            rhs=w1_sb[:, h, fc, :],
                            start=(fc == 0), stop=(fc == FC - 1),
                        )
                    nc.scalar.copy(wh_sb[:, c, :], aug[:])
                    # e2 column -> DRAM row segment
                    nc.sync.dma_start(
                        e2row[c * 128:(c + 1) * 128],
                        wh_sb[:, c, O + 1:O + 2].bitcast(f32),
                    )
                # broadcast e2 row to all partitions
                e2bc = hpool.tile([128, N], f32)
                e2r_ap = e2row[:]
                nc.sync.dma_start(
                    e2bc[:],
                    bass.AP(tensor=e2r_ap.tensor, offset=e2r_ap.offset,
                            ap=[[0, 128], [1, N]]),
                )
                # e = leaky(e1 + e2 + mneg); p = exp(e) with row-sum
                p_sb = hpool.tile([128, NC, N], f32)
                zt = hpool.tile([128, NC, N], f32)
                em = hpool.tile([128, NC, N], f32)
                zsum = hpool.tile([128, NC], f32)
                zinv = hpool.tile([128, NC], f32)
                for c in range(NC):
                    nc.vector.tensor_add(em[:, c, :], e2bc[:], mneg[:, c, :])
                    nc.scalar.activation(
                        zt[:, c, :], em[:, c, :],
                        mybir.ActivationFunctionType.Identity,
                        bias=wh_sb[:, c, O:O + 1].bitcast(f32),
                    )
                    nc.vector.tensor_scalar_mul(em[:, c, :], zt[:, c, :], ALPHA)
                    nc.vector.tensor_max(zt[:, c, :], zt[:, c, :], em[:, c, :])
                    nc.scalar.activation(
                        p_sb[:, c, :], zt[:, c, :],
                        mybir.ActivationFunctionType.Exp,
                        accum_out=zsum[:, c:c + 1],
                    )
                nc.vector.reciprocal(zinv[:], zsum[:])
                for c in range(NC):
                    nc.vector.tensor_scalar_mul(
                        p_sb[:, c, :], p_sb[:, c, :], zinv[:, c:c + 1]
                    )
                # transpose attn: pT[m, n]
                pT = hpool.tile([128, NC, N], mm_dt)
                for c in range(NC):
                    for d in range(NC):
                        tp = ps_tr.tile([128, 128], f32, tag="tr")
                        nc.tensor.transpose(
                            tp[:], p_sb[:, c, d * 128:(d + 1) * 128], ident[:]
                        )
                        nc.vector.tensor_copy(
                            pT[:, d, c * 128:(c + 1) * 128], tp[:]
                        )
                # out1T[o, n] = Wh.T @ pT ; elu -> hT rows
                for oc in range(NC):
                    ops = ps_big.tile([128, N], f32, tag="big")
                    for mc in range(NC):
                        nc.tensor.matmul(
                            ops[:], lhsT=wh_sb[:, mc, oc * 128:(oc + 1) * 128],
                            rhs=pT[:, mc, :],
                            start=(mc == 0), stop=(mc == NC - 1),
                        )
                    at = hpool.tile([128, N], f32)
                    rt = hpool.tile([128, N], f32)
                    nc.scalar.activation(at[:], ops[:],
                                         mybir.ActivationFunctionType.Exp)
                    nc.scalar.activation(rt[:], ops[:],
                                         mybir.ActivationFunctionType.Relu)
                    nc.vector.tensor_scalar(
                        at[:], at[:], 1.0, 0.0,
                        op0=mybir.AluOpType.subtract, op1=mybir.AluOpType.min,
                    )
                    nc.vector.tensor_add(hT[:, h * NC + oc, :], at[:], rt[:])

            # ---- layer 2 (single head, OUT wide) ----
            wh2_sb = gpool.tile([128, NC, OUT], mm_dt)
            e12 = gpool.tile([128, NC, 2], f32)
            e2row2 = drp.tile([N], f32, tag="e2row2")
            for c in range(NC):
                w2ps = ps_big.tile([128, OUT], f32, tag="big")
                aug2 = ps_aug.tile([128, 2], f32, tag="aug")
                for k in range(KC2):
                    nc.tensor.matmul(
                        w2ps[:], lhsT=hT[:, k, c * 128:(c + 1) * 128],
                        rhs=w2_sb[:, k, 0:OUT],
                        start=(k == 0), stop=(k == KC2 - 1),
                    )
                for k in range(KC2):
                    nc.tensor.matmul(
                        aug2[:], lhsT=hT[:, k, c * 128:(c + 1) * 128],
                        rhs=w2_sb[:, k, OUT:OUT + 2],
                        start=(k == 0), stop=(k == KC2 - 1),
                    )
                # fold non_pad_mask into Wh2 and e1/e2
                nc.scalar.activation(
                    wh2_sb[:, c, :], w2ps[:],
                    mybir.ActivationFunctionType.Copy,
                    scale=npm_sb[:, c:c + 1],
                )
                nc.scalar.activation(
                    e12[:, c, :], aug2[:],
                    mybir.ActivationFunctionType.Copy,
                    scale=npm_sb[:, c:c + 1],
                )
                nc.sync.dma_start(
                    e2row2[c * 128:(c + 1) * 128], e12[:, c, 1:2]
                )
            e2bc2 = gpool.tile([128, N], f32)
            e2r2_ap = e2row2[:]
            nc.sync.dma_start(
                e2bc2[:],
                bass.AP(tensor=e2r2_ap.tensor, offset=e2r2_ap.offset,
                        ap=[[0, 128], [1, N]]),
            )
            p2 = gpool.tile([128, NC, N], f32)
            z2t = gpool.tile([128, NC, N], f32)
            em2 = gpool.tile([128, NC, N], f32)
            z2sum = gpool.tile([128, NC], f32)
            z2inv = gpool.tile([128, NC], f32)
            sc2 = gpool.tile([128, NC], f32)
            for c in range(NC):
                nc.vector.tensor_add(em2[:, c, :], e2bc2[:], mneg[:, c, :])
                nc.scalar.activation(
                    z2t[:, c, :], em2[:, c, :],
                    mybir.ActivationFunctionType.Identity,
                    bias=e12[:, c, 0:1],
                )
                nc.vector.tensor_scalar_mul(em2[:, c, :], z2t[:, c, :], ALPHA)
                nc.vector.tensor_max(z2t[:, c, :], z2t[:, c, :], em2[:, c, :])
                nc.scalar.activation(
                    p2[:, c, :], z2t[:, c, :],
                    mybir.ActivationFunctionType.Exp,
                    accum_out=z2sum[:, c:c + 1],
                )
            nc.vector.reciprocal(z2inv[:], z2sum[:])
            nc.vector.tensor_mul(sc2[:], z2inv[:], npm_sb[:])
            pT2 = gpool.tile([128, NC, N], mm_dt)
            for c in range(NC):
                for d in range(NC):
                    tp = ps_tr.tile([128, 128], f32, tag="tr")
                    nc.tensor.transpose(
                        tp[:], p2[:, c, d * 128:(d + 1) * 128], ident[:]
                    )
                    nc.vector.tensor_copy(pT2[:, d, c * 128:(c + 1) * 128], tp[:])
            out_sb = gpool.tile([128, NC, OUT], f32)
            for c in range(NC):
                o2ps = ps_big.tile([128, OUT], f32, tag="big")
                for mc in range(NC):
                    nc.tensor.matmul(
                        o2ps[:], lhsT=pT2[:, mc, c * 128:(c + 1) * 128],
                        rhs=wh2_sb[:, mc, :],
                        start=(mc == 0), stop=(mc == NC - 1),
                    )
                a2 = gpool.tile([128, OUT], f32)
                r2 = gpool.tile([128, OUT], f32)
                nc.scalar.activation(a2[:], o2ps[:],
                                     mybir.ActivationFunctionType.Exp,
                                     scale=sc2[:, c:c + 1])
                nc.scalar.activation(r2[:], o2ps[:],
                                     mybir.ActivationFunctionType.Relu,
                                     scale=sc2[:, c:c + 1])
                nc.vector.tensor_scalar(
                    a2[:], a2[:], 1.0, 0.0,
                    op0=mybir.AluOpType.subtract, op1=mybir.AluOpType.min,
                )
                nc.vector.tensor_add(out_sb[:, c, :], a2[:], r2[:])
            nc.sync.dma_start(out_d[g], out_sb[:])

    nc.compile()
    return nc


_NC_CACHE = {}


def build_kernel(mm_dt=MM_DT):
    key = str(mm_dt)
    if key not in _NC_CACHE:
        _NC_CACHE[key] = _build_nc(mm_dt)
    return _NC_CACHE[key]


def _host_prep(fea, adj, non_pad_mask, embed, W_heads, a_heads, W_out, a_out):
    """Fold attention vectors into weights (f64) and pre-layout per-core inputs."""
    W64 = W_heads.astype(np.float64)
    w1 = np.einsum("hfo,ho->hf", W64, a_heads[:, :O].astype(np.float64))
    w2 = np.einsum("hfo,ho->hf", W64, a_heads[:, O:].astype(np.float64))
    # (H, F, O+2) -> pad F to 384 -> (H, 3, 128, O+2)
    w1aug = np.concatenate(
        [W_heads.astype(np.float64), w1[:, :, None], w2[:, :, None]], axis=2
    )
    w1aug = np.pad(w1aug, ((0, 0), (0, 384 - F), (0, 0))).astype(np.float32)
    w1aug = w1aug.reshape(H, FC, 128, O + 2)

    Wo64 = W_out.astype(np.float64)
    w1o = Wo64 @ a_out[:OUT].astype(np.float64)
    w2o = Wo64 @ a_out[OUT:].astype(np.float64)
    w2aug = np.concatenate(
        [Wo64, w1o[:, None], w2o[:, None]], axis=1
    ).astype(np.float32).reshape(KC2, 128, OUT + 2)

    emb_pad = np.zeros((VC * 128, 384), np.float32)
    emb_pad[:VOCAB, :F] = embed

    # one-hot: [b, p, vc, n] = (fea[b, n] == vc*128 + p)
    vidx = np.arange(VC * 128).reshape(VC, 128)
    oh = (fea[:, None, None, :] == vidx[None, :, :, None])  # (B, VC, 128, N)
    oh = np.ascontiguousarray(
        oh.transpose(0, 2, 1, 3)).astype(np.float32)        # (B, 128, VC, N)

    adjm = np.ascontiguousarray(
        adj.reshape(B, NC, 128, N).transpose(0, 2, 1, 3)).astype(np.int32)
    npm = np.ascontiguousarray(
        non_pad_mask.reshape(B, NC, 128).transpose(0, 2, 1)).astype(np.float32)

    emb_l = np.ascontiguousarray(emb_pad.reshape(VC, 128, 384))
    return oh, adjm, npm, emb_l, w1aug, w2aug


def kernel(fea, adj, non_pad_mask, embed, W_heads, a_heads, W_out, a_out,
           _mm_dt=None, _trace=False):
    mm_dt = MM_DT if _mm_dt is None else _mm_dt
    oh, adjm, npm, emb_l, w1aug, w2aug = _host_prep(
        fea, adj, non_pad_mask, embed, W_heads, a_heads, W_out, a_out)

    nc = build_kernel(mm_dt)
    identity = np.eye(128, dtype=np.float32)
    in_maps = []
    for i in range(NCORES):
        sl = slice(i * GPC, (i + 1) * GPC)
        in_maps.append({
            "oh": oh[sl], "adjm": adjm[sl], "npm": npm[sl],
            "emb": emb_l, "w1aug": w1aug, "w2aug": w2aug,
            "identity": identity,
        })
    res = run_bass_kernel_spmd(nc, in_maps, core_ids=list(range(NCORES)),
                               trace=_trace)
    # out_d is [GPC, 128, NC, OUT] per core -> (B, N, OUT)
    outs = []
    for i in range(NCORES):
        o = res.results[i]["out"]                   # (GPC, 128, NC, OUT)
        outs.append(o.transpose(0, 2, 1, 3).reshape(GPC, N, OUT))
    full = np.concatenate(outs, axis=0).astype(np.float32)
    if _trace:
        kernel.last_results = res
    return full


# revision 17
# speedup vs baseline: 1.8696x; 1.8696x over previous
"""GAT (2-layer graph attention network) Trainium2 kernel.

Contract: kernel(**inputs) takes the FULL inputs from setup_inputs() and
returns the full (32, 256, 512) float32 output. Internally shards the batch
across 8 NeuronCores (4 graphs per core), runs a Bass/Tile kernel per core,
and concatenates the results.

Math notes (reference in reference.py):
  x = embed[fea]                       -> computed on device as onehot @ embed
  Layer1 per head h: Wh = x @ W[h];  e1 = Wh @ a1 = x @ (W[h] @ a1)
    so [Wh | e1] come from ONE matmul with rhs [W[h] | W@a1]; the e2 vectors
    for ALL heads come from one skinny matmul (W@a2 stacked, M=8) against xT,
    then are broadcast across partitions with a ones(1,128) matmul into PSUM.
  e = leaky_relu(e1[:,None] + e2[None,:], 0.2); mask; softmax; out = attn@Wh.
  Softmax skips max-subtraction: e is O(1) for real rows; masked entries get
  -1000 added pre-leaky so exp(0.2*(-1000+e)) underflows to 0, matching
  where(mask, e, -9e15).
  Layer1 output is produced o-major (Wh.T @ pT), which is exactly the hT
  layout layer 2 needs for its stationary operand - no h transpose.
  non_pad_mask folds into layer-2's PSUM->SBUF copy scale and the final
  activation scale (multiplying rows of a matmul output == masking its input).
"""

import numpy as np
from contextlib import ExitStack

import concourse.bass as bass
import concourse.tile as tile
from concourse import mybir, bacc
from concourse.bass_utils import run_bass_kernel_spmd

f32 = mybir.dt.float32
i32 = mybir.dt.int32
AF = mybir.ActivationFunctionType
AL = mybir.AluOpType

# Problem dims (hardcoded per contract)
B, N, VOCAB, F, O, H, OUT = 32, 256, 200, 300, 256, 8, 512
NCORES = 8
GPC = B // NCORES          # graphs per core
NC = N // 128              # node chunks (2)
FC = 3                     # feature chunks (F padded 300->384)
VC = 2                     # vocab chunks (padded 200->256)
KC2 = (H * O) // 128       # layer-2 contraction chunks (16)
ALPHA = 0.2
MASK_NEG = -1000.0

# Matmul dtype: float32 (exact, 4 cyc/row) or float32r (tf32-like, 1 cyc/row)
MM_DT = mybir.dt.float32r


def _build_nc(mm_dt):
    nc = bacc.Bacc("TRN2", target_bir_lowering=False, debug=False,
                   num_devices=NCORES)

    oh_d = nc.dram_tensor("oh", [GPC, 128, VC, N], f32, kind="ExternalInput").ap()
    adj_d = nc.dram_tensor("adjm", [GPC, 128, NC, N], i32, kind="ExternalInput").ap()
    npm_d = nc.dram_tensor("npm", [GPC, 128, NC], f32, kind="ExternalInput").ap()
    emb_d = nc.dram_tensor("emb", [128, VC, 384], f32, kind="ExternalInput").ap()
    w1_d = nc.dram_tensor("w1aug", [128, H, FC, O + 2], f32, kind="ExternalInput").ap()
    w2c_d = nc.dram_tensor("w2cols", [128, FC, H], f32, kind="ExternalInput").ap()
    wo_d = nc.dram_tensor("woaug", [128, KC2, OUT + 2], f32, kind="ExternalInput").ap()
    idn_d = nc.dram_tensor("identity", [128, 128], f32, kind="ExternalInput").ap()
    out_d = nc.dram_tensor("out", [GPC, 128, NC, OUT], f32, kind="ExternalOutput").ap()

    with tile.TileContext(nc) as tc, ExitStack() as ctx:
        const = ctx.enter_context(tc.tile_pool(name="const", bufs=1))
        gpool = ctx.enter_context(tc.tile_pool(name="gpool", bufs=2))
        hpool = ctx.enter_context(tc.tile_pool(name="hpool", bufs=3))
        hbig = ctx.enter_context(tc.tile_pool(name="hbig", bufs=2))
        ps_aug = ctx.enter_context(tc.tile_pool(name="ps_aug", bufs=2, space="PSUM"))
        ps_big = ctx.enter_context(tc.tile_pool(name="ps_big", bufs=2, space="PSUM"))
        ps_tr = ctx.enter_context(tc.tile_pool(name="ps_tr", bufs=2, space="PSUM"))
        ps_bc = ctx.enter_context(tc.tile_pool(name="ps_bc", bufs=2, space="PSUM"))

        # ---- resident constants ----
        ident = const.tile([128, 128], f32)
        nc.sync.dma_start(ident[:], idn_d)
        ones_f = const.tile([1, 128], f32)
        nc.vector.memset(ones_f[:], 1.0)

        stage = ctx.enter_context(tc.tile_pool(name="stage", bufs=3))

        def staged(dst_ap, src_ap, nfree):
            # stage f32 DMA -> compute-copy so the tile is a "rounded" f32r
            # producer; dst/src must be 2D-viewable as (128, nfree)
            if mm_dt == f32:
                nc.sync.dma_start(dst_ap, src_ap)
            else:
                st = stage.tile([128, 900], f32, tag="st")
                nc.sync.dma_start(st[:, :nfree], src_ap)
                nc.vector.tensor_copy(dst_ap, st[:, :nfree])

        emb_sb = const.tile([128, VC, 384], mm_dt)
        for c in range(VC):
            staged(emb_sb[:, c, :], emb_d[:, c, :], 384)
        w1_sb = const.tile([128, H, FC, O + 2], mm_dt)
        for h in range(H):
            staged(w1_sb[:, h].rearrange("p a b -> p (a b)"),
                   w1_d[:, h].rearrange("p a b -> p (a b)"), FC * (O + 2))
        w2c_sb = const.tile([128, FC, H], mm_dt)
        staged(w2c_sb[:].rearrange("p a b -> p (a b)"),
               w2c_d[:].rearrange("p a b -> p (a b)"), FC * H)
        wo_sb = const.tile([128, KC2, OUT + 2], mm_dt)
        for k in range(KC2):
            staged(wo_sb[:, k], wo_d[:, k], OUT + 2)
        ones_sb = const.tile([1, 128], mm_dt)
        nc.vector.tensor_copy(ones_sb[:], ones_f[:])

        for g in range(GPC):
            # ---- per-graph inputs ----
            oh_f = gpool.tile([128, VC, N], f32)
            nc.sync.dma_start(oh_f[:], oh_d[g])
            if mm_dt == f32:
                oh_sb = oh_f
            else:
                oh_sb = gpool.tile([128, VC, N], mm_dt)
                nc.vector.tensor_copy(oh_sb[:], oh_f[:])
            adj_sb = gpool.tile([128, NC, N], i32)
            nc.sync.dma_start(adj_sb[:], adj_d[g])
            npm_sb = gpool.tile([128, NC], f32)
            nc.sync.dma_start(npm_sb[:], npm_d[g])
            # mneg[n, m] = 0 where edge, MASK_NEG where not
            mneg = gpool.tile([128, NC, N], f32)
            nc.vector.tensor_scalar(
                mneg[:], adj_sb[:], 1.0, -MASK_NEG, op0=AL.subtract, op1=AL.mult)

            # ---- xT = (onehot @ embed).T : [f, n] ----
            xT = gpool.tile([128, FC, N], mm_dt)
            for fc in range(FC):
                ps = ps_big.tile([128, N], f32, tag="big")
                for vc in range(VC):
                    nc.tensor.matmul(
                        ps[:], lhsT=emb_sb[:, vc, fc * 128:(fc + 1) * 128],
                        rhs=oh_sb[:, vc, :], start=(vc == 0), stop=(vc == VC - 1))
                nc.scalar.copy(xT[:, fc, :], ps[:])

            # ---- e2 rows for all heads: (8, N) = w2cols.T @ xT ----
            e2ps = ps_aug.tile([8, N], f32, tag="aug")
            for fc in range(FC):
                nc.tensor.matmul(e2ps[:], lhsT=w2c_sb[:, fc, :], rhs=xT[:, fc, :],
                                 start=(fc == 0), stop=(fc == FC - 1))
            e2all = gpool.tile([8, N], f32)
            nc.vector.tensor_copy(e2all[:], e2ps[:])

            # ---- layer 1 heads -> hT [hf, n] o-major ----
            hT = hbig.tile([128, KC2, N], mm_dt)
            for h in range(H):
                # augmented matmul [Wh | e1] per node chunk
                wh_sb = hpool.tile([128, NC, O + 2], mm_dt)
                for c in range(NC):
                    aug = ps_aug.tile([128, O + 2], f32, tag="aug")
                    for fc in range(FC):
                        nc.tensor.matmul(
                            aug[:], lhsT=xT[:, fc, c * 128:(c + 1) * 128],
                            rhs=w1_sb[:, h, fc, :],
                            start=(fc == 0), stop=(fc == FC - 1))
                    nc.vector.tensor_copy(wh_sb[:, c, :], aug[:])
                # broadcast e2 row of head h to 128 partitions (PSUM)
                # move head's e2 row to partition 0 (DMA crosses partitions),
                # round, then broadcast via ones-matmul into PSUM
                e2row_f = hpool.tile([1, N], f32, tag="e2rf")
                nc.sync.dma_start(e2row_f[:], e2all[h:h + 1, :])
                e2row_r = hpool.tile([1, N], mm_dt, tag="e2rr")
                nc.vector.tensor_copy(e2row_r[:], e2row_f[:])
                e2bc = ps_bc.tile([128, N], f32, tag="bc")
                nc.tensor.matmul(e2bc[:], lhsT=ones_sb[:], rhs=e2row_r[:],
                                 start=True, stop=True)
                # softmax(leaky(e1 + e2 + mask))
                p_sb = hpool.tile([128, NC, N], f32)
                em = hpool.tile([128, NC, N], f32)
                zt = hpool.tile([128, NC, N], f32)
                zsum = hpool.tile([128, NC], f32)
                zinv = hpool.tile([128, NC], f32)
                for c in range(NC):
                    e1col = wh_sb[:, c, O:O + 1].bitcast(f32)
                    nc.vector.tensor_add(em[:, c, :], e2bc[:], mneg[:, c, :])
                    nc.vector.tensor_scalar(
                        zt[:, c, :], em[:, c, :], e1col, None, op0=AL.add)
                    nc.vector.tensor_scalar(
                        em[:, c, :], em[:, c, :], e1col, ALPHA,
                        op0=AL.add, op1=AL.mult)
                    nc.vector.tensor_tensor(
                        zt[:, c, :], zt[:, c, :], em[:, c, :], op=AL.max)
                    nc.scalar.activation(
                        p_sb[:, c, :], zt[:, c, :], AF.Exp,
                        accum_out=zsum[:, c:c + 1])
                nc.vector.reciprocal(zinv[:], zsum[:])
                for c in range(NC):
                    nc.vector.tensor_scalar_mul(
                        p_sb[:, c, :], p_sb[:, c, :], zinv[:, c:c + 1])
                # transpose p into one PSUM bank, single copy out
                tp4 = ps_tr.tile([128, NC * NC, 128], f32, tag="tr")
                for c in range(NC):
                    for d in range(NC):
                        nc.tensor.transpose(
                            tp4[:, c * NC + d, :],
                            p_sb[:, c, d * 128:(d + 1) * 128], ident[:])
                pT = hpool.tile([128, NC, N], mm_dt)
                nc.vector.tensor_copy(
                    pT[:].rearrange("p d (c u) -> p c d u", u=128),
                    tp4[:].rearrange("p (c d) u -> p c d u", d=NC))
                # out1T[o, n] = Wh.T @ pT ; elu -> hT rows
                for oc in range(NC):
                    ops = ps_big.tile([128, N], f32, tag="big")
                    for mc in range(NC):
                        nc.tensor.matmul(
                            ops[:], lhsT=wh_sb[:, mc, oc * 128:(oc + 1) * 128],
                            rhs=pT[:, mc, :], start=(mc == 0), stop=(mc == NC - 1))
                    at = hpool.tile([128, N], f32)
                    rt = hpool.tile([128, N], f32)
                    nc.scalar.activation(at[:], ops[:], AF.Exp)
                    nc.vector.tensor_scalar(rt[:], ops[:], 0.0, None, op0=AL.max)
                    nc.vector.tensor_scalar(
                        at[:], at[:], 1.0, 0.0, op0=AL.subtract, op1=AL.min)
                    nc.vector.tensor_add(hT[:, h * NC + oc, :], at[:], rt[:])

            # ---- layer 2 ----
            wh2_sb = gpool.tile([128, NC, OUT], mm_dt)
            e12 = gpool.tile([128, NC, 2], f32)
            for c in range(NC):
                w2ps = ps_big.tile([128, OUT], f32, tag="big")
                aug2 = ps_aug.tile([128, 2], f32, tag="aug")
                for k in range(KC2):
                    nc.tensor.matmul(
                        w2ps[:], lhsT=hT[:, k, c * 128:(c + 1) * 128],
                        rhs=wo_sb[:, k, 0:OUT],
                        start=(k == 0), stop=(k == KC2 - 1))
                for k in range(KC2):
                    nc.tensor.matmul(
                        aug2[:], lhsT=hT[:, k, c * 128:(c + 1) * 128],
                        rhs=wo_sb[:, k, OUT:OUT + 2],
                        start=(k == 0), stop=(k == KC2 - 1))
                nc.scalar.activation(wh2_sb[:, c, :], w2ps[:], AF.Copy,
                                     scale=npm_sb[:, c:c + 1])
                nc.scalar.activation(e12[:, c, :], aug2[:], AF.Copy,
                                     scale=npm_sb[:, c:c + 1])
            # e2 row via PE transpose of the two column chunks, then broadcast
            e2r_ps = ps_aug.tile([1, N], f32, tag="aug")
            for c in range(NC):
                nc.tensor.transpose(e2r_ps[:, c * 128:(c + 1) * 128],
                                    e12[:, c, 1:2], ident[:])
            e2row2 = gpool.tile([1, N], mm_dt)
            nc.vector.tensor_copy(e2row2[:], e2r_ps[:])
            e2bc2 = ps_bc.tile([128, N], f32, tag="bc")
            nc.tensor.matmul(e2bc2[:], lhsT=ones_sb[:], rhs=e2row2[:],
                             start=True, stop=True)
            p2 = hpool.tile([128, NC, N], f32, tag="p_sb")
            em2 = hpool.tile([128, NC, N], f32, tag="em")
            z2t = hpool.tile([128, NC, N], f32, tag="zt")
            z2sum = gpool.tile([128, NC], f32)
            z2inv = gpool.tile([128, NC], f32)
            sc2 = gpool.tile([128, NC], f32)
            for c in range(NC):
                e1col = e12[:, c, 0:1]
                nc.vector.tensor_add(em2[:, c, :], e2bc2[:], mneg[:, c, :])
                nc.vector.tensor_scalar(
                    z2t[:, c, :], em2[:, c, :], e1col, None, op0=AL.add)
                nc.vector.tensor_scalar(
                    em2[:, c, :], em2[:, c, :], e1col, ALPHA,
                    op0=AL.add, op1=AL.mult)
                nc.vector.tensor_tensor(
                    z2t[:, c, :], z2t[:, c, :], em2[:, c, :], op=AL.max)
                nc.scalar.activation(p2[:, c, :], z2t[:, c, :], AF.Exp,
                                     accum_out=z2sum[:, c:c + 1])
            nc.vector.reciprocal(z2inv[:], z2sum[:])
            nc.vector.tensor_mul(sc2[:], z2inv[:], npm_sb[:])
            tp4b = ps_tr.tile([128, NC * NC, 128], f32, tag="tr")
            for c in range(NC):
                for d in range(NC):
                    nc.tensor.transpose(tp4b[:, c * NC + d, :],
                                        p2[:, c, d * 128:(d + 1) * 128], ident[:])
            pT2 = hpool.tile([128, NC, N], mm_dt, tag="pT")
            nc.vector.tensor_copy(
                pT2[:].rearrange("p d (c u) -> p c d u", u=128),
                tp4b[:].rearrange("p (c d) u -> p c d u", d=NC))
            out_sb = gpool.tile([128, NC, OUT], f32)
            for c in range(NC):
                o2ps = ps_big.tile([128, OUT], f32, tag="big")
                for mc in range(NC):
                    nc.tensor.matmul(
                        o2ps[:], lhsT=pT2[:, mc, c * 128:(c + 1) * 128],
                        rhs=wh2_sb[:, mc, :], start=(mc == 0), stop=(mc == NC - 1))
                a2 = gpool.tile([128, OUT], f32)
                r2 = gpool.tile([128, OUT], f32)
                nc.scalar.activation(a2[:], o2ps[:], AF.Exp,
                                     scale=sc2[:, c:c + 1])
                nc.vector.tensor_scalar(
                    r2[:], o2ps[:], sc2[:, c:c + 1], 0.0,
                    op0=AL.mult, op1=AL.max)
                nc.vector.tensor_scalar(
                    a2[:], a2[:], 1.0, 0.0, op0=AL.subtract, op1=AL.min)
                nc.vector.tensor_add(out_sb[:, c, :], a2[:], r2[:])
            nc.sync.dma_start(out_d[g], out_sb[:])

    nc.compile()
    return nc


_NC_CACHE = {}


def build_kernel(mm_dt=MM_DT):
    key = str(mm_dt)
    if key not in _NC_CACHE:
        _NC_CACHE[key] = _build_nc(mm_dt)
    return _NC_CACHE[key]


def _host_prep(fea, adj, non_pad_mask, embed, W_heads, a_heads, W_out, a_out):
    """Fold attention vectors into weights (f64) and pre-layout per-core inputs."""
    W64 = W_heads.astype(np.float64)
    w1 = np.einsum("hfo,ho->hf", W64, a_heads[:, :O].astype(np.float64))
    w2 = np.einsum("hfo,ho->hf", W64, a_heads[:, O:].astype(np.float64))
    # w1aug: [128, H, FC, O+1] p-major
    w1aug_full = np.concatenate(
        [W64, w1[:, :, None], np.zeros((H, F, 1))], axis=2)     # (H, F, O+2)
    w1aug_full = np.pad(w1aug_full, ((0, 0), (0, 384 - F), (0, 0)))
    w1aug = np.ascontiguousarray(
        w1aug_full.reshape(H, FC, 128, O + 2).transpose(2, 0, 1, 3)
    ).astype(np.float32)
    # w2cols: [128, FC, H]
    w2p = np.pad(w2, ((0, 0), (0, 384 - F)))                     # (H, 384)
    w2cols = np.ascontiguousarray(
        w2p.reshape(H, FC, 128).transpose(2, 1, 0)).astype(np.float32)

    Wo64 = W_out.astype(np.float64)
    w1o = Wo64 @ a_out[:OUT].astype(np.float64)
    w2o = Wo64 @ a_out[OUT:].astype(np.float64)
    woaug = np.concatenate([Wo64, w1o[:, None], w2o[:, None]], axis=1)
    woaug = np.ascontiguousarray(
        woaug.reshape(KC2, 128, OUT + 2).transpose(1, 0, 2)).astype(np.float32)

    emb_pad = np.zeros((VC * 128, 384), np.float32)
    emb_pad[:VOCAB, :F] = embed
    emb_l = np.ascontiguousarray(
        emb_pad.reshape(VC, 128, 384).transpose(1, 0, 2))

    vidx = np.arange(VC * 128).reshape(VC, 128)
    oh = (fea[:, None, None, :] == vidx[None, :, :, None])       # (B, VC, 128, N)
    oh = np.ascontiguousarray(oh.transpose(0, 2, 1, 3)).astype(np.float32)

    adjm = np.ascontiguousarray(
        adj.reshape(B, NC, 128, N).transpose(0, 2, 1, 3)).astype(np.int32)
    npm = np.ascontiguousarray(
        non_pad_mask.reshape(B, NC, 128).transpose(0, 2, 1)).astype(np.float32)

    return oh, adjm, npm, emb_l, w1aug, w2cols, woaug


def kernel(fea, adj, non_pad_mask, embed, W_heads, a_heads, W_out, a_out,
           _mm_dt=None, _trace=False):
    mm_dt = MM_DT if _mm_dt is None else _mm_dt
    oh, adjm, npm, emb_l, w1aug, w2cols, woaug = _host_prep(
        fea, adj, non_pad_mask, embed, W_heads, a_heads, W_out, a_out)

    nc = build_kernel(mm_dt)
    identity = np.eye(128, dtype=np.float32)
    in_maps = []
    for i in range(NCORES):
        sl = slice(i * GPC, (i + 1) * GPC)
        in_maps.append({
            "oh": oh[sl], "adjm": adjm[sl], "npm": npm[sl],
            "emb": emb_l, "w1aug": w1aug, "w2cols": w2cols, "woaug": woaug,
            "identity": identity,
        })
    res = run_bass_kernel_spmd(nc, in_maps, core_ids=list(range(NCORES)),
                               trace=_trace)
    outs = []
    for i in range(NCORES):
        o = res.results[i]["out"]                   # (GPC, 128, NC, OUT)
        outs.append(o.transpose(0, 2, 1, 3).reshape(GPC, N, OUT))
    full = np.concatenate(outs, axis=0).astype(np.float32)
    if _trace:
        kernel.last_results = res
    return full
